# revision 15
# baseline (speedup 1.0000x reference)
"""Cross-attention kernel for Trainium2, sharded over 8 NeuronCores.

Problem (hardcoded): b=4, n=m=2048, query_dim=context_dim=512,
heads=8, dim_head=64 (inner=512), f32 I/O.

Sharding: data-parallel over (batch, query-half): core c -> batch c//2,
query rows [(c%2)*1024, (c%2+1)*1024). Each core holds the full K/V
context for its batch, so there are no collectives and output shards
tile the full output exactly.

Design (ScalarE exp on the full score matrix is the binding floor;
everything else is arranged to hide under it):
  - Scores use 2x row-tiled matmuls (tile_position (0,0)/(64,0) derived
    from base partitions): head pair (2p, 2p+1) shares one PE pass, so
    the k=64 contraction no longer wastes half the array. kT2 holds each
    pair's K^T stacked [64+64, M]; measured on HW: the second matmul of
    a pair retires ~4ns after the first.
  - Attention runs in (pair, nj) passes of 512 query columns; per m-tile
    one [128,1024] PSUM tile holds both heads' scores and is consumed by
    a single 1024-wide exp. PSUM: 2x2 scores banks + 3 attn-out banks +
    1 projection/output bank = 8.
  - V is stored per m-chunk as [128, h, 128] with columns 64:128 all
    ones: the attn@V matmul then replicates the softmax denominator
    across PSUM partitions 64:128 for free. Each finished attn-out bank
    is evacuated with one [128,512] copy; reciprocal+multiply run
    deferred, a quarter at a time, dripped into later iterations (the
    custom-DVE reciprocal_approx_fast is broken on this HW - measured
    garbage - so this uses the exact DVE reciprocal off the hot path).
  - All projection work except a 4-chunk warmup is interleaved into the
    attention loop (PE issue order per iteration: scores(mi), proj,
    attnV(mi-1), so the FIFO never queues work behind a pending exp).
  - The first 4 output-projection n-tiles run inside the final pass;
    the rest interleave with the final normalizations in the tail.
  - Inputs stream on two HWDGE queues (sync + scalar) in first-use
    order so the warmup chunks start after ~1.5MB, not 7MB.
"""

import numpy as np
import ml_dtypes

import concourse.bass as bass
import concourse.mybir as mybir
import concourse.tile as tile
from concourse import bacc
from concourse.bass_utils import run_bass_kernel_spmd

BF16 = mybir.dt.bfloat16
F32 = mybir.dt.float32

B, N, M = 4, 2048, 2048
CDIM, INNER = 512, 512
H, D = 8, 64
NSH = N // 2  # query rows per core
N_CORES = 8
SCALE = D ** -0.5

CC = CDIM // 128   # contraction chunks for projections (4)
IC = INNER // 128  # inner-dim chunks == head pairs (4)
MT = M // 128      # m tiles (16)
NJ = NSH // 512    # n passes of 512 (2)
NT = NSH // 128    # n tiles (8)
MJ = M // 512      # m chunks of 512 (4)

ROW_TILED = True


def build_nc() -> bass.Bass:
    nc = bacc.Bacc(None)

    pixelT = nc.dram_tensor("pixelT", [CDIM, NSH], BF16, kind="ExternalInput")
    patchT = nc.dram_tensor("patchT", [CDIM, M], BF16, kind="ExternalInput")
    wq = nc.dram_tensor("wq", [CDIM, INNER], BF16, kind="ExternalInput")
    wk = nc.dram_tensor("wk", [CDIM, INNER], BF16, kind="ExternalInput")
    wv = nc.dram_tensor("wv", [CDIM, INNER], BF16, kind="ExternalInput")
    wo = nc.dram_tensor("wo", [INNER, CDIM], BF16, kind="ExternalInput")
    bo = nc.dram_tensor("bo", [CDIM], F32, kind="ExternalInput")
    out = nc.dram_tensor("out", [NSH, CDIM], F32, kind="ExternalOutput")

    with tile.TileContext(nc) as tc:
        with (
            tc.tile_pool(name="weights", bufs=1) as wpool,
            tc.tile_pool(name="acts", bufs=1) as apool,
            tc.tile_pool(name="qkv", bufs=1) as qkvpool,
            tc.tile_pool(name="vsb", bufs=MT) as vpool,
            tc.tile_pool(name="attn", bufs=6) as attnpool,
            tc.tile_pool(name="small", bufs=4) as rpool,
            tc.tile_pool(name="stage", bufs=3) as stpool,
        ):
            # ---- inputs, two HWDGE queues, first-use order ----------------
            wq_sb = wpool.tile([128, CC, INNER], BF16, tag="wq")
            wk_sb = wpool.tile([128, CC, INNER], BF16, tag="wk")
            wv_sb = wpool.tile([128, CC, INNER], BF16, tag="wv")
            wo_sb = wpool.tile([128, IC, CDIM], BF16, tag="wo")
            pixT = apool.tile([128, CC, NSH], BF16, tag="pixT")
            patT = apool.tile([128, CC, M], BF16, tag="patT")

            pix_r = pixelT.rearrange("(cc p) n -> p cc n", p=128)
            pat_r = patchT.rearrange("(cc p) m -> p cc m", p=128)

            # three parallel DMA streams, each in first-use order
            # scalar (ACT) HWDGE queue: Q-side then V weights
            nc.scalar.dma_start(wq_sb, wq.rearrange("(cc p) i -> p cc i", p=128))
            for cc in range(CC):
                nc.scalar.dma_start(pixT[:, cc, :], pix_r[:, cc, :])
            nc.scalar.dma_start(wv_sb, wv.rearrange("(cc p) i -> p cc i", p=128))
            # sync HWDGE queue: K weights + first context half
            nc.sync.dma_start(wk_sb, wk.rearrange("(cc p) i -> p cc i", p=128))
            for cc in range(CC):
                nc.sync.dma_start(patT[:, cc, 0:1024], pat_r[:, cc, 0:1024])
            # gpsimd SWDGE queue: second context half + tail weights
            for cc in range(CC):
                nc.gpsimd.dma_start(patT[:, cc, 1024:2048], pat_r[:, cc, 1024:2048])
            nc.gpsimd.dma_start(wo_sb, wo.rearrange("(ic p) o -> p ic o", p=128))
            bo_sb = wpool.tile([128, CDIM], F32, tag="bo")
            nc.sync.dma_start(
                bo_sb,
                bass.AP(tensor=bo[:].tensor, offset=0, ap=[[0, 128], [1, CDIM]]),
            )

            qT = qkvpool.tile([128, IC, NSH], BF16, tag="qT")
            if ROW_TILED:
                # kT2: per pair p, rows 0:64 = head 2p K^T, 64:128 = head 2p+1
                kT2 = qkvpool.tile([128, IC, M], BF16, tag="kT2")
            else:
                kT2 = qkvpool.tile([128, IC, 2, M], BF16, tag="kT2")
                for ic in range(IC):
                    nc.vector.memset(kT2[D : 2 * D, ic, 0, :], 0.0)
                    nc.vector.memset(kT2[0:D, ic, 1, :], 0.0)
            # v_sb: [m-chunk 128, head, 128] = [V_h | ones]: cols 64:128 make
            # the attn@V matmul replicate the softmax denominator over
            # PSUM partitions 64:128.
            v_sb = [
                vpool.tile([128, H, 128], BF16, tag="v", name=f"v{mi}")
                for mi in range(MT)
            ]
            for mi in range(MT):
                nc.vector.memset(v_sb[mi][:, :, D:128], 1.0)

            # warm the exp table early so the first real exp isn't gated on it
            warm = rpool.tile([1, 16], BF16, tag="warm")
            nc.scalar.activation(
                warm, pixT[0:1, 0, 0:16], mybir.ActivationFunctionType.Exp
            )

            # ---- projection chunk emitters --------------------------------
            def q_chunk(pool, ic, nj):
                ps = pool.tile([128, 512], F32, tag="pp", name=f"q{ic}_{nj}")
                sl = slice(nj * 512, (nj + 1) * 512)
                for cc in range(CC):
                    nc.tensor.matmul(
                        ps,
                        wq_sb[:, cc, ic * 128 : (ic + 1) * 128],
                        pixT[:, cc, sl],
                        start=(cc == 0),
                        stop=(cc == CC - 1),
                    )
                nc.vector.tensor_copy(qT[:, ic, sl], ps)

            def k_chunk(pool, ic, mj):
                ps = pool.tile([128, 512], F32, tag="pp", name=f"k{ic}_{mj}")
                sl = slice(mj * 512, (mj + 1) * 512)
                for cc in range(CC):
                    nc.tensor.matmul(
                        ps,
                        wk_sb[:, cc, ic * 128 : (ic + 1) * 128],
                        patT[:, cc, sl],
                        start=(cc == 0),
                        stop=(cc == CC - 1),
                    )
                if ROW_TILED:
                    nc.vector.tensor_copy(kT2[:, ic, sl], ps)
                else:
                    nc.vector.tensor_copy(kT2[0:D, ic, 0, sl], ps[0:D, :])
                    nc.vector.tensor_copy(
                        kT2[D : 2 * D, ic, 1, sl], ps[D : 2 * D, :]
                    )

            def v_chunk(pool, mi):
                ps = pool.tile([128, 512], F32, tag="pp", name=f"v{mi}")
                sl = slice(mi * 128, (mi + 1) * 128)
                for cc in range(CC):
                    nc.tensor.matmul(
                        ps,
                        patT[:, cc, sl],
                        wv_sb[:, cc, :],
                        start=(cc == 0),
                        stop=(cc == CC - 1),
                    )
                nc.vector.tensor_copy(
                    v_sb[mi][:, :, 0:D], ps.rearrange("p (h d) -> p h d", h=H)
                )

            def emit_chunk(pool, item):
                kind, a, b = item
                if kind == "v":
                    v_chunk(pool, a)
                elif kind == "k":
                    k_chunk(pool, a, b)
                else:
                    q_chunk(pool, a, b)

            # ---- upfront: the bare minimum to start pair 0 ----------------
            with tc.tile_pool(name="ppsum", bufs=2, space="PSUM") as pp:
                q_chunk(pp, 0, 0)
                k_chunk(pp, 0, 0)
                v_chunk(pp, 0)
                v_chunk(pp, 1)

            # remaining projection chunks keyed by global attention iteration
            proj_sched = {}
            proj_sched[0] = [("k", 0, 1), ("v", 2, 0)]
            proj_sched[1] = [("k", 0, 2), ("v", 3, 0)]
            proj_sched[2] = [("k", 0, 3), ("v", 4, 0)]
            for i, mi in enumerate(range(5, MT)):
                proj_sched[3 + i] = [("v", mi, 0)]          # iters 3..13
            proj_sched[10].append(("q", 0, 1))
            # K/Q for later pairs: every 3rd iteration so DVE keeps up with
            # the deferred norms in the leftover slots (Q before the pair's
            # last K chunks so qT is ready a pass early)
            tail_items = []
            for ic in range(1, IC):
                tail_items += [
                    ("k", ic, 0), ("k", ic, 1), ("q", ic, 0),
                    ("k", ic, 2), ("k", ic, 3), ("q", ic, 1),
                ]
            for i, item in enumerate(tail_items):
                proj_sched[16 + 3 * i] = [item]

            # ---- attention ------------------------------------------------
            outT = qkvpool.tile([128, IC, NSH], BF16, tag="outT")
            # deferred normalization: (oc tile, outT row offset, pair, nj, q)
            norm_q = []

            def norm_item():
                oc, po, p_, nj_, q = norm_q.pop(0)
                qsl = slice(nj_ * 512 + q * 128, nj_ * 512 + (q + 1) * 128)
                csl = slice(q * 128, (q + 1) * 128)
                r64 = rpool.tile([D, 128], F32, tag="r64")
                nc.vector.reciprocal(r64, oc[D : 2 * D, csl])
                nc.vector.tensor_mul(outT[po : po + D, p_, qsl], oc[0:D, csl], r64)

            def emit_out(ni, pool):
                ps = pool.tile([128, CDIM], F32, tag="pp", name=f"f{ni}")
                for ic in range(IC):
                    nc.tensor.matmul(
                        ps,
                        outT[:, ic, ni * 128 : (ni + 1) * 128],
                        wo_sb[:, ic, :],
                        start=(ic == 0),
                        stop=(ic == IC - 1),
                    )
                st = stpool.tile([128, CDIM], F32, tag="st", name=f"st{ni}")
                nc.vector.tensor_add(st, ps, bo_sb)
                eng = nc.sync if ni % 2 == 0 else nc.scalar
                eng.dma_start(out[ni * 128 : (ni + 1) * 128, :], st)

            with (
                tc.tile_pool(name="spsum", bufs=2, space="PSUM") as spool,
                tc.tile_pool(name="opsum", bufs=3, space="PSUM") as opool,
                tc.tile_pool(name="jpsum", bufs=1, space="PSUM") as jpool,
            ):
                giter = 0
                for p in range(IC):
                    for nj in range(NJ):
                        last_pass = p == IC - 1 and nj == NJ - 1
                        njsl = slice(nj * 512, (nj + 1) * 512)
                        o_he = opool.tile([128, 512], F32, tag="o", name=f"oh{p}{nj}")
                        o_ho = opool.tile([128, 512], F32, tag="o", name=f"ol{p}{nj}")
                        at_prev = None
                        for mi in range(MT):
                            misl = slice(mi * 128, (mi + 1) * 128)
                            s = spool.tile([128, 1024], F32, tag="s")
                            if ROW_TILED:
                                nc.tensor.matmul(
                                    s[:, 0:512],
                                    kT2[0:D, p, misl],
                                    qT[0:D, p, njsl],
                                    start=True,
                                    stop=True,
                                )
                                nc.tensor.matmul(
                                    s[:, 512:1024],
                                    kT2[D : 2 * D, p, misl],
                                    qT[D : 2 * D, p, njsl],
                                    start=True,
                                    stop=True,
                                )
                            else:
                                nc.tensor.matmul(
                                    s[:, 0:512],
                                    kT2[:, p, 0, misl],
                                    qT[:, p, njsl],
                                    start=True,
                                    stop=True,
                                )
                                nc.tensor.matmul(
                                    s[:, 512:1024],
                                    kT2[:, p, 1, misl],
                                    qT[:, p, njsl],
                                    start=True,
                                    stop=True,
                                )
                            # fill PE slack under the exp stream
                            if giter in proj_sched:
                                for item in proj_sched.pop(giter):
                                    emit_chunk(jpool, item)
                            elif norm_q:
                                norm_item()
                            if last_pass and mi in (8, 10, 12, 14):
                                emit_out((mi - 8) // 2, jpool)
                            # attn@V for the previous m-tile (software
                            # pipeline: never queue PE work behind an exp
                            # that has not run yet)
                            if at_prev is not None:
                                nc.tensor.matmul(
                                    o_he,
                                    v_sb[mi - 1][:, 2 * p, :],
                                    at_prev[:, 0:512],
                                    start=(mi == 1),
                                    stop=False,
                                )
                                nc.tensor.matmul(
                                    o_ho,
                                    v_sb[mi - 1][:, 2 * p + 1, :],
                                    at_prev[:, 512:1024],
                                    start=(mi == 1),
                                    stop=False,
                                )
                            at = attnpool.tile([128, 1024], BF16, tag="at")
                            nc.scalar.activation(
                                at, s, mybir.ActivationFunctionType.Exp, scale=SCALE
                            )
                            at_prev = at
                            giter += 1
                        # drain the pipeline: last m-tile's attn@V
                        nc.tensor.matmul(
                            o_he,
                            v_sb[MT - 1][:, 2 * p, :],
                            at_prev[:, 0:512],
                            start=False,
                            stop=True,
                        )
                        nc.tensor.matmul(
                            o_ho,
                            v_sb[MT - 1][:, 2 * p + 1, :],
                            at_prev[:, 512:1024],
                            start=False,
                            stop=True,
                        )
                        # evacuate both banks fast (rows 0:64 head output,
                        # rows 64:128 replicated denominator); the
                        # reciprocal+mul run deferred via norm_q.
                        ocs = []
                        for nm, o_ps in (("he", o_he), ("ho", o_ho)):
                            oc = stpool.tile(
                                [128, 512], F32, tag="oc", bufs=6,
                                name=f"oc{nm}{p}{nj}",
                            )
                            nc.vector.tensor_copy(oc, o_ps)
                            ocs.append(oc)
                        for q in range(4):
                            norm_q.append((ocs[0], 0, p, nj, q))
                            norm_q.append((ocs[1], D, p, nj, q))

                # tail: final pass normalizations + remaining output tiles
                for q in range(4):
                    norm_item()
                    norm_item()
                    emit_out(4 + q, jpool)

    nc.finalize()
    return nc


def make_in_maps(pixel_embed, patch_embed, Wq, Wk, Wv, Wo, bo):
    bf = ml_dtypes.bfloat16
    pixel_embed = np.asarray(pixel_embed, dtype=np.float32)
    patch_embed = np.asarray(patch_embed, dtype=np.float32)
    wq = np.asarray(Wq, dtype=np.float32).astype(bf)
    wk = np.asarray(Wk, dtype=np.float32).astype(bf)
    wv = np.asarray(Wv, dtype=np.float32).astype(bf)
    wo = np.asarray(Wo, dtype=np.float32).astype(bf)
    bo = np.asarray(bo, dtype=np.float32)

    in_maps = []
    for core in range(N_CORES):
        bi, half = divmod(core, 2)
        px = pixel_embed[bi, half * NSH : (half + 1) * NSH, :]  # [NSH, CDIM]
        pa = patch_embed[bi]  # [M, CDIM]
        in_maps.append(
            {
                "pixelT": px.T.astype(bf),
                "patchT": pa.T.astype(bf),
                "wq": wq,
                "wk": wk,
                "wv": wv,
                "wo": wo,
                "bo": bo,
            }
        )
    return in_maps


def gather_out(results):
    out = np.empty((B, N, CDIM), np.float32)
    for core in range(N_CORES):
        bi, half = divmod(core, 2)
        out[bi, half * NSH : (half + 1) * NSH, :] = results[core]["out"]
    return out


_NC_CACHE = {}


def kernel(pixel_embed, patch_embed, Wq, Wk, Wv, Wo, bo, **kw):
    if "nc" not in _NC_CACHE:
        _NC_CACHE["nc"] = build_nc()
    nc = _NC_CACHE["nc"]
    in_maps = make_in_maps(pixel_embed, patch_embed, Wq, Wk, Wv, Wo, bo)
    res = run_bass_kernel_spmd(nc, in_maps, core_ids=list(range(N_CORES)), **kw)
    out = gather_out(res.results)
    if kw.get("trace"):
        return out, res
    return out


# revision 18
# speedup vs baseline: 1.0235x; 1.0235x over previous
"""Cross-attention kernel for Trainium2, sharded over 8 NeuronCores.

Problem (hardcoded): b=4, n=m=2048, query_dim=context_dim=512,
heads=8, dim_head=64 (inner=512), f32 I/O.

Sharding: data-parallel over (batch, query-half): core c -> batch c//2,
query rows [(c%2)*1024, (c%2+1)*1024). Each core holds the full K/V
context for its batch, so there are no collectives and output shards
tile the full output exactly.

Design (ScalarE exp on the full score matrix is the binding floor;
everything else is arranged to hide under it):
  - Scores use 2x row-tiled matmuls (tile_position (0,0)/(64,0) derived
    from base partitions): head pair (2p, 2p+1) shares one PE pass, so
    the k=64 contraction no longer wastes half the array. kT2 holds each
    pair's K^T stacked [64+64, M]; measured on HW: the second matmul of
    a pair retires ~4ns after the first.
  - Attention runs in (pair, nj) passes of 512 query columns; per m-tile
    one [128,1024] PSUM tile holds both heads' scores and is consumed by
    a single 1024-wide exp. PSUM: 2x2 scores banks + 3 attn-out banks +
    1 projection/output bank = 8.
  - V is stored per m-chunk as [128, h, 128] with columns 64:128 all
    ones: the attn@V matmul then replicates the softmax denominator
    across PSUM partitions 64:128 for free. Each finished attn-out bank
    is evacuated with one [128,512] copy; reciprocal+multiply run
    deferred, a quarter at a time, dripped into later iterations (the
    custom-DVE reciprocal_approx_fast is broken on this HW - measured
    garbage - so this uses the exact DVE reciprocal off the hot path).
  - All projection work except a 4-chunk warmup is interleaved into the
    attention loop (PE issue order per iteration: scores(mi), proj,
    attnV(mi-1), so the FIFO never queues work behind a pending exp).
  - The first 4 output-projection n-tiles run inside the final pass;
    the rest interleave with the final normalizations in the tail.
  - Inputs stream on two HWDGE queues (sync + scalar) in first-use
    order so the warmup chunks start after ~1.5MB, not 7MB.
"""

import numpy as np
import ml_dtypes

import concourse.bass as bass
import concourse.mybir as mybir
import concourse.tile as tile
from concourse import bacc
from concourse.bass_utils import run_bass_kernel_spmd

BF16 = mybir.dt.bfloat16
F32 = mybir.dt.float32

B, N, M = 4, 2048, 2048
CDIM, INNER = 512, 512
H, D = 8, 64
NSH = N // 2  # query rows per core
N_CORES = 8
SCALE = D ** -0.5

CC = CDIM // 128   # contraction chunks for projections (4)
IC = INNER // 128  # inner-dim chunks == head pairs (4)
MT = M // 128      # m tiles (16)
NJ = NSH // 512    # n passes of 512 (2)
NT = NSH // 128    # n tiles (8)
MJ = M // 512      # m chunks of 512 (4)

ROW_TILED = True


def build_nc() -> bass.Bass:
    nc = bacc.Bacc(None)

    pixelT = nc.dram_tensor("pixelT", [CDIM, NSH], BF16, kind="ExternalInput")
    patchT = nc.dram_tensor("patchT", [CDIM, M], BF16, kind="ExternalInput")
    wq = nc.dram_tensor("wq", [CDIM, INNER], BF16, kind="ExternalInput")
    wk = nc.dram_tensor("wk", [CDIM, INNER], BF16, kind="ExternalInput")
    wv = nc.dram_tensor("wv", [CDIM, INNER], BF16, kind="ExternalInput")
    wo = nc.dram_tensor("wo", [INNER, CDIM], BF16, kind="ExternalInput")
    bo = nc.dram_tensor("bo", [CDIM], F32, kind="ExternalInput")
    out = nc.dram_tensor("out", [NSH, CDIM], F32, kind="ExternalOutput")

    with tile.TileContext(nc) as tc:
        with (
            tc.tile_pool(name="weights", bufs=1) as wpool,
            tc.tile_pool(name="acts", bufs=1) as apool,
            tc.tile_pool(name="qkv", bufs=1) as qkvpool,
            tc.tile_pool(name="vsb", bufs=MT) as vpool,
            tc.tile_pool(name="attn", bufs=6) as attnpool,
            tc.tile_pool(name="small", bufs=4) as rpool,
            tc.tile_pool(name="stage", bufs=3) as stpool,
        ):
            # ---- inputs, two HWDGE queues, first-use order ----------------
            wq_sb = wpool.tile([128, CC, INNER], BF16, tag="wq")
            wk_sb = wpool.tile([128, CC, INNER], BF16, tag="wk")
            wv_sb = wpool.tile([128, CC, INNER], BF16, tag="wv")
            wo_sb = wpool.tile([128, IC, CDIM], BF16, tag="wo")
            pixT = apool.tile([128, CC, NSH], BF16, tag="pixT")
            patT = apool.tile([128, CC, M], BF16, tag="patT")

            pix_r = pixelT.rearrange("(cc p) n -> p cc n", p=128)
            pat_r = patchT.rearrange("(cc p) m -> p cc m", p=128)

            # three parallel DMA streams, each in first-use order
            # scalar (ACT) HWDGE queue: Q-side then V weights
            nc.scalar.dma_start(wq_sb, wq.rearrange("(cc p) i -> p cc i", p=128))
            for cc in range(CC):
                nc.scalar.dma_start(pixT[:, cc, :], pix_r[:, cc, :])
            nc.scalar.dma_start(wv_sb, wv.rearrange("(cc p) i -> p cc i", p=128))
            # sync HWDGE queue: K weights + context in consumption order
            nc.sync.dma_start(wk_sb, wk.rearrange("(cc p) i -> p cc i", p=128))
            for mj in range(MJ):
                sl = slice(mj * 512, (mj + 1) * 512)
                for cc in range(CC):
                    nc.sync.dma_start(patT[:, cc, sl], pat_r[:, cc, sl])
            bo_sb = wpool.tile([128, CDIM], F32, tag="bo")
            nc.sync.dma_start(
                bo_sb,
                bass.AP(tensor=bo[:].tensor, offset=0, ap=[[0, 128], [1, CDIM]]),
            )
            nc.gpsimd.dma_start(wo_sb, wo.rearrange("(ic p) o -> p ic o", p=128))

            qT = qkvpool.tile([128, IC, NSH], BF16, tag="qT")
            if ROW_TILED:
                # kT2: per pair p, rows 0:64 = head 2p K^T, 64:128 = head 2p+1
                kT2 = qkvpool.tile([128, IC, M], BF16, tag="kT2")
            else:
                kT2 = qkvpool.tile([128, IC, 2, M], BF16, tag="kT2")
                for ic in range(IC):
                    nc.vector.memset(kT2[D : 2 * D, ic, 0, :], 0.0)
                    nc.vector.memset(kT2[0:D, ic, 1, :], 0.0)
            # v_sb: [m-chunk 128, head, 128] = [V_h | ones]: cols 64:128 make
            # the attn@V matmul replicate the softmax denominator over
            # PSUM partitions 64:128.
            v_sb = [
                vpool.tile([128, H, 128], BF16, tag="v", name=f"v{mi}")
                for mi in range(MT)
            ]
            for mi in range(MT):
                nc.vector.memset(v_sb[mi][:, :, D:128], 1.0)

            # warm the exp table early so the first real exp isn't gated on it
            warm = rpool.tile([1, 16], BF16, tag="warm")
            nc.scalar.activation(
                warm, pixT[0:1, 0, 0:16], mybir.ActivationFunctionType.Exp
            )

            # ---- projection chunk emitters --------------------------------
            def q_chunk(pool, ic, nj):
                ps = pool.tile([128, 512], F32, tag="pp", name=f"q{ic}_{nj}")
                sl = slice(nj * 512, (nj + 1) * 512)
                for cc in range(CC):
                    nc.tensor.matmul(
                        ps,
                        wq_sb[:, cc, ic * 128 : (ic + 1) * 128],
                        pixT[:, cc, sl],
                        start=(cc == 0),
                        stop=(cc == CC - 1),
                    )
                nc.vector.tensor_copy(qT[:, ic, sl], ps)

            def k_chunk(pool, ic, mj):
                ps = pool.tile([128, 512], F32, tag="pp", name=f"k{ic}_{mj}")
                sl = slice(mj * 512, (mj + 1) * 512)
                for cc in range(CC):
                    nc.tensor.matmul(
                        ps,
                        wk_sb[:, cc, ic * 128 : (ic + 1) * 128],
                        patT[:, cc, sl],
                        start=(cc == 0),
                        stop=(cc == CC - 1),
                    )
                if ROW_TILED:
                    nc.vector.tensor_copy(kT2[:, ic, sl], ps)
                else:
                    nc.vector.tensor_copy(kT2[0:D, ic, 0, sl], ps[0:D, :])
                    nc.vector.tensor_copy(
                        kT2[D : 2 * D, ic, 1, sl], ps[D : 2 * D, :]
                    )

            def v_chunk(pool, mi):
                ps = pool.tile([128, 512], F32, tag="pp", name=f"v{mi}")
                sl = slice(mi * 128, (mi + 1) * 128)
                for cc in range(CC):
                    nc.tensor.matmul(
                        ps,
                        patT[:, cc, sl],
                        wv_sb[:, cc, :],
                        start=(cc == 0),
                        stop=(cc == CC - 1),
                    )
                nc.vector.tensor_copy(
                    v_sb[mi][:, :, 0:D], ps.rearrange("p (h d) -> p h d", h=H)
                )

            def emit_chunk(pool, item):
                kind, a, b = item
                if kind == "v":
                    v_chunk(pool, a)
                elif kind == "k":
                    k_chunk(pool, a, b)
                else:
                    q_chunk(pool, a, b)

            # ---- upfront: the bare minimum to start pair 0 ----------------
            with tc.tile_pool(name="ppsum", bufs=2, space="PSUM") as pp:
                q_chunk(pp, 0, 0)
                k_chunk(pp, 0, 0)
                v_chunk(pp, 0)
                v_chunk(pp, 1)

            # remaining projection chunks keyed by global attention
            # iteration. Pass 0 (iters 0..13): V just-in-time + K-ic0 spaced
            # to DMA arrival. Later passes: local even slots 2..12 so the
            # two iterations around each pass boundary keep the DVE queue
            # empty (the attn-out bank evacuation copies must run at once).
            proj_sched = {g: [("v", g + 2, 0)] for g in range(MT - 2)}
            proj_sched[0].insert(0, ("k", 0, 1))
            proj_sched[4].insert(0, ("k", 0, 2))
            proj_sched[8].insert(0, ("k", 0, 3))
            proj_sched[13].append(("q", 0, 1))
            for ic in range(1, IC):
                base = 16 * ic
                items = [("k", ic, 0), ("k", ic, 1), ("k", ic, 2),
                         ("k", ic, 3), ("q", ic, 0), ("q", ic, 1)]
                for i, item in enumerate(items):
                    proj_sched[base + 2 + 2 * i] = [item]

            # ---- attention ------------------------------------------------
            outT = qkvpool.tile([128, IC, NSH], BF16, tag="outT")
            # deferred normalization: (oc tile, outT row offset, pair, nj, q)
            norm_q = []

            def norm_item():
                oc, po, p_, nj_, q = norm_q.pop(0)
                qsl = slice(nj_ * 512 + q * 128, nj_ * 512 + (q + 1) * 128)
                csl = slice(q * 128, (q + 1) * 128)
                r64 = rpool.tile([D, 128], F32, tag="r64")
                nc.vector.reciprocal(r64, oc[D : 2 * D, csl])
                nc.vector.tensor_mul(outT[po : po + D, p_, qsl], oc[0:D, csl], r64)

            def emit_out(ni, pool):
                ps = pool.tile([128, CDIM], F32, tag="pp", name=f"f{ni}")
                for ic in range(IC):
                    nc.tensor.matmul(
                        ps,
                        outT[:, ic, ni * 128 : (ni + 1) * 128],
                        wo_sb[:, ic, :],
                        start=(ic == 0),
                        stop=(ic == IC - 1),
                    )
                st = stpool.tile([128, CDIM], F32, tag="st", name=f"st{ni}")
                nc.vector.tensor_add(st, ps, bo_sb)
                eng = nc.sync if ni % 2 == 0 else nc.scalar
                eng.dma_start(out[ni * 128 : (ni + 1) * 128, :], st)

            with (
                tc.tile_pool(name="spsum", bufs=2, space="PSUM") as spool,
                tc.tile_pool(name="opsum", bufs=3, space="PSUM") as opool,
                tc.tile_pool(name="jpsum", bufs=1, space="PSUM") as jpool,
            ):
                NPASS = IC * NJ
                GT = NPASS * MT

                def pass_of(g):
                    P, mi = divmod(g, MT)
                    return P // NJ, P % NJ, mi

                def finish_pass(o_he, o_ho, p, nj):
                    # evacuate both banks with one copy each (rows 0:64 head
                    # output, rows 64:128 replicated denominator); the
                    # reciprocal+mul run deferred via norm_q.
                    ocs = []
                    for nm, o_ps in (("he", o_he), ("ho", o_ho)):
                        oc = stpool.tile(
                            [128, 512], F32, tag="oc", bufs=6,
                            name=f"oc{nm}{p}{nj}",
                        )
                        nc.vector.tensor_copy(oc, o_ps)
                        ocs.append(oc)
                    for q in range(4):
                        norm_q.append((ocs[0], 0, p, nj, q))
                        norm_q.append((ocs[1], D, p, nj, q))

                o_cur = o_prev = None
                at_prev = None
                for g in range(GT + 1):
                    if g < GT:
                        p, nj, mi = pass_of(g)
                        njsl = slice(nj * 512, (nj + 1) * 512)
                        misl = slice(mi * 128, (mi + 1) * 128)
                        if mi == 0:
                            o_prev = o_cur
                            o_cur = (
                                opool.tile([128, 512], F32, tag="o", name=f"oh{p}{nj}"),
                                opool.tile([128, 512], F32, tag="o", name=f"ol{p}{nj}"),
                            )
                        s = spool.tile([128, 1024], F32, tag="s")
                        if ROW_TILED:
                            nc.tensor.matmul(
                                s[:, 0:512],
                                kT2[0:D, p, misl],
                                qT[0:D, p, njsl],
                                start=True,
                                stop=True,
                            )
                            nc.tensor.matmul(
                                s[:, 512:1024],
                                kT2[D : 2 * D, p, misl],
                                qT[D : 2 * D, p, njsl],
                                start=True,
                                stop=True,
                            )
                        else:
                            nc.tensor.matmul(
                                s[:, 0:512],
                                kT2[:, p, 0, misl],
                                qT[:, p, njsl],
                                start=True,
                                stop=True,
                            )
                            nc.tensor.matmul(
                                s[:, 512:1024],
                                kT2[:, p, 1, misl],
                                qT[:, p, njsl],
                                start=True,
                                stop=True,
                            )
                    # attn@V for the previous global iteration (software
                    # pipeline: never queue PE work behind a pending exp;
                    # crosses pass boundaries without a bubble)
                    if at_prev is not None:
                        pp_, pnj_, pmi = pass_of(g - 1)
                        oh, ol = o_cur if pmi < MT - 1 or g == GT else o_prev
                        if pmi == MT - 1 and g < GT:
                            oh, ol = o_prev
                        nc.tensor.matmul(
                            oh,
                            v_sb[pmi][:, 2 * pp_, :],
                            at_prev[:, 0:512],
                            start=(pmi == 0),
                            stop=(pmi == MT - 1),
                        )
                        nc.tensor.matmul(
                            ol,
                            v_sb[pmi][:, 2 * pp_ + 1, :],
                            at_prev[:, 512:1024],
                            start=(pmi == 0),
                            stop=(pmi == MT - 1),
                        )
                        if pmi == MT - 1:
                            finish_pass(oh, ol, pp_, pnj_)
                    if g < GT:
                        # fill PE slack under the exp stream; emitted after the
                        # attn@V + bank-evacuation block so boundary copies
                        # stay at the front of the DVE queue
                        if g in proj_sched:
                            for item in proj_sched.pop(g):
                                emit_chunk(jpool, item)
                        elif norm_q and mi < MT - 2:
                            norm_item()
                        if g >= 7 * MT and mi in (8, 10, 12, 14):
                            emit_out((mi - 8) // 2, jpool)
                        at = attnpool.tile([128, 1024], BF16, tag="at")
                        nc.scalar.activation(
                            at, s, mybir.ActivationFunctionType.Exp, scale=SCALE
                        )
                        at_prev = at

                # tail: final pass normalizations + remaining output tiles
                for q in range(4):
                    norm_item()
                    norm_item()
                    emit_out(4 + q, jpool)

    nc.finalize()
    return nc


def make_in_maps(pixel_embed, patch_embed, Wq, Wk, Wv, Wo, bo):
    bf = ml_dtypes.bfloat16
    pixel_embed = np.asarray(pixel_embed, dtype=np.float32)
    patch_embed = np.asarray(patch_embed, dtype=np.float32)
    wq = np.asarray(Wq, dtype=np.float32).astype(bf)
    wk = np.asarray(Wk, dtype=np.float32).astype(bf)
    wv = np.asarray(Wv, dtype=np.float32).astype(bf)
    wo = np.asarray(Wo, dtype=np.float32).astype(bf)
    bo = np.asarray(bo, dtype=np.float32)

    in_maps = []
    for core in range(N_CORES):
        bi, half = divmod(core, 2)
        px = pixel_embed[bi, half * NSH : (half + 1) * NSH, :]  # [NSH, CDIM]
        pa = patch_embed[bi]  # [M, CDIM]
        in_maps.append(
            {
                "pixelT": px.T.astype(bf),
                "patchT": pa.T.astype(bf),
                "wq": wq,
                "wk": wk,
                "wv": wv,
                "wo": wo,
                "bo": bo,
            }
        )
    return in_maps


def gather_out(results):
    out = np.empty((B, N, CDIM), np.float32)
    for core in range(N_CORES):
        bi, half = divmod(core, 2)
        out[bi, half * NSH : (half + 1) * NSH, :] = results[core]["out"]
    return out


_NC_CACHE = {}


def kernel(pixel_embed, patch_embed, Wq, Wk, Wv, Wo, bo, **kw):
    if "nc" not in _NC_CACHE:
        _NC_CACHE["nc"] = build_nc()
    nc = _NC_CACHE["nc"]
    in_maps = make_in_maps(pixel_embed, patch_embed, Wq, Wk, Wv, Wo, bo)
    res = run_bass_kernel_spmd(nc, in_maps, core_ids=list(range(N_CORES)), **kw)
    out = gather_out(res.results)
    if kw.get("trace"):
        return out, res
    return out
